# revision 5
# baseline (speedup 1.0000x reference)
"""Trainium2 Bass kernel for a 2-layer Elman RNN decoder — k-sharded
partial-sum version (ReduceScatter instead of AllGather).

Math per step (B=64, H=4761->K=4768, T=128):
    h0 = tanh(xproj + h0 @ W_hh0.T)
    h1 = tanh(b1 + h0 @ W_ih1.T + h1 @ W_hh1.T)

Sharding: CONTRACTION-parallel. Core m owns hidden components
k in [596m, 596m+596) (padded to KS=640 rows). It holds W[:, k-slice]
(shipped as [KS, 4768]) for all three weights and its OWN h-shard as the
matmul stationary. Per app it computes partial[b, j] over its k-slice for
ALL j=4768 outputs, folds psum to f32, and a ReduceScatter(add) over the
8 cores hands core m the fully-summed j-slice [64, 596] — which after
tanh IS its next h-shard. No AllGather, no DRAM stationary reload: the
collective output is 8x smaller than the gathered alternative and the
next stationary is produced locally by PE-transposing the core's own
shard.

Precision scheme (per weight W, scaled by SW=512):
    W*SW = Whi + Wlo     fp16 hi + fp16 UNSCALED residual (SW chosen so
                         the residual ~|W|*SW*2^-11 ~ 5e-3 stays in fp16
                         normal range)
    h    = hhi + hlo/SLH fp16 pair, SLH=4096
Stationary per k-tile = [hlo | hhi] (128 PE cols). Two moving passes share
one stationary load:
    pass1 (moving Whi): psum[0:64] += hlo.Whi ; psum[64:128] += hhi.Whi
    pass2 (moving Wlo): psum[0:64] += hlo.Wlo ; psum[64:128] += hhi.Wlo
=> psum[64:128] = hhi.(W*SW), psum[0:64] = hlo.(W*SW); the fold computes
v = psum[64:128] + psum[0:64]/SLH (exact algebra, no dropped cross term).
Layer-1's two products (h1.Whh1 and h0new.Wih1) accumulate into the SAME
psum chunks; the W_hh1 half runs early to cover the layer-0
ReduceScatter latency.
"""

import os
import numpy as np

import concourse.bass as bass
import concourse.bacc as bacc
import concourse.tile as tile
from concourse import mybir
from concourse.bass_utils import run_bass_kernel_spmd

H_REAL = 4761
K = 4768            # padded hidden size (8 * 596)
B = 64              # batch
B2 = 2 * B          # lo|hi stationary width per k-tile
T_FULL = 128
NCORES = 8
J = K // NCORES     # 596: j-slice (RS output) and real k-slice per core
KS = 640            # k-slice padded to 5*128
KT_S = KS // 128    # 5 k-tiles per core
SW = 512.0          # weight scale (power of 2)
SLH = 4096.0        # h lo-part scale (power of 2)
F16 = mybir.dt.float16
F32 = mybir.dt.float32

_ALU = mybir.AluOpType

# output j-chunks per half-app: psum bank limit is 512 f32
_HALF = K // 2      # 2384
_CHUNKS = [(0, 512), (512, 1024), (1024, 1536), (1536, 2048), (2048, _HALF)]


def _group_splits(a, b):
    """Split global j-range [a,b) at 596-group boundaries -> (g, glo, ghi, s)."""
    out = []
    g = a // J
    while 596 * g < b:
        lo = max(a, J * g)
        hi = min(b, J * (g + 1))
        out.append((g, lo - J * g, hi - J * g, lo - a, hi - a))
        g += 1
    return out


def build(tc, outs, ins, T, T_out=None):
    nc = tc.nc
    ys = outs["ys"]
    T_out = T if T_out is None else T_out

    import contextlib
    with contextlib.ExitStack() as ctx:
        wpool = ctx.enter_context(tc.tile_pool(name="wpool", bufs=1))
        hstpool = ctx.enter_context(tc.tile_pool(name="hstpool", bufs=2))
        cpool = ctx.enter_context(tc.tile_pool(name="cpool", bufs=1))
        lopool = ctx.enter_context(tc.tile_pool(name="lopool", bufs=3))
        fpool = ctx.enter_context(tc.tile_pool(name="fpool", bufs=2))
        vpool = ctx.enter_context(tc.tile_pool(name="vpool", bufs=3))
        hpool = ctx.enter_context(tc.tile_pool(name="hpool", bufs=2))
        pc_pool = ctx.enter_context(tc.tile_pool(name="pcpool", bufs=1, space="PSUM"))
        tp_pool = ctx.enter_context(tc.tile_pool(name="tppool", bufs=2, space="PSUM"))
        dpool = ctx.enter_context(tc.tile_pool(name="dpool", bufs=2, space="DRAM"))

        # ---- constants ----
        ident = cpool.tile([128, 128], F16)
        nc.sync.dma_start(ident, ins["ident"])
        xproj_sb = cpool.tile([B, J], F32)
        nc.sync.dma_start(xproj_sb, ins["xproj"])
        b1_sb = cpool.tile([B, J], F32)
        nc.sync.dma_start(b1_sb, ins["b1"])


        # ---- resident hi weights: [KS, 4768] -> [128, KT_S*4768] ----
        def load_k_rows(dst_sb, src_dram, n):
            nc.sync.dma_start(
                dst_sb.rearrange("p (kt x) -> p kt x", x=n),
                src_dram.rearrange("(kt p) x -> p kt x", p=128))

        w_hi_sb = {}
        for wname in ("w0", "w1h", "w1i"):
            wsb = wpool.tile([128, KT_S * K], F16, name=f"{wname}_hi_sb")
            load_k_rows(wsb, ins[f"{wname}_hi"], K)
            w_hi_sb[wname] = wsb

        # ---- stationary h-shards: [128, KT_S * (lo|hi)] ----
        def new_stationary(prefix):
            return hstpool.tile([128, KT_S * B2], F16, name=f"{prefix}st",
                                tag=f"{prefix}st")

        h0st = new_stationary("h0")
        h1st = new_stationary("h1")
        load_k_rows(h0st, ins["h0t"], B2)
        load_k_rows(h1st, ins["h1t"], B2)

        def stream_lo(wname, half, kt):
            """One k-tile x half-width of the lo residual: [128, 2384]."""
            lo_t = lopool.tile([128, _HALF], F16, name="lo_t", tag="lo")
            nc.sync.dma_start(
                lo_t, ins[f"{wname}_lo"][128 * kt:128 * (kt + 1),
                                         _HALF * half:_HALF * (half + 1)])
            return lo_t

        def mm_half(ps_chunks, wname, hst, half, first, last):
            """All k-tiles x 5 chunks of one half-width app into held psums."""
            whi = w_hi_sb[wname]
            for kt in range(KT_S):
                lo_t = stream_lo(wname, half, kt)
                st = hst[0:128, kt * B2:(kt + 1) * B2]
                f = first and kt == 0
                l = last and kt == KT_S - 1
                for ci, (c0, c1) in enumerate(_CHUNKS):
                    cw = c1 - c0
                    gc = _HALF * half + c0
                    nc.tensor.matmul(
                        ps_chunks[ci][0:128, 0:cw], st,
                        whi[:, kt * K + gc:kt * K + gc + cw],
                        start=f, stop=False, skip_group_check=True)
                    nc.tensor.matmul(
                        ps_chunks[ci][0:128, 0:cw], st,
                        lo_t[:, c0:c1],
                        start=False, stop=l, skip_group_check=True)

        def fold_half(ps_chunks, half, v_dram):
            for ci, (c0, c1) in enumerate(_CHUNKS):
                cw = c1 - c0
                ps = ps_chunks[ci]
                tx = fpool.tile([B, 512], F32, name="tx", tag="fx")
                nc.vector.tensor_scalar_mul(tx[:, 0:cw], ps[0:B, 0:cw],
                                            1.0 / SLH)
                v = vpool.tile([B, 512], F32, name="v", tag="v")
                nc.vector.tensor_tensor(v[:, 0:cw], tx[:, 0:cw],
                                        ps[B:128, 0:cw], _ALU.add)
                a = _HALF * half + c0
                for g, glo, ghi, slo, shi in _group_splits(a, a + cw):
                    nc.sync.dma_start(
                        v_dram[B * g:B * (g + 1), glo:ghi], v[:, slo:shi])

        def apply_rs(tagp, mk_half):
            """Run both halves through mm+fold, then ReduceScatter."""
            v_dram = dpool.tile([NCORES * B, J], F32, name="vd",
                                tag=f"vd{tagp}")
            for half in (0, 1):
                ps_chunks = [pc_pool.tile([128, 512], F32, name=f"pc{ci}",
                                          tag=f"pc{ci % 5}")
                             for ci in range(5)]
                mk_half(ps_chunks, half)
                fold_half(ps_chunks, half, v_dram)
            rs = dpool.tile([B, J], F32, name="rs", tag=f"rs{tagp}")
            nc.gpsimd.collective_compute(
                "ReduceScatter", _ALU.add,
                replica_groups=[list(range(NCORES))],
                ins=[v_dram.opt()], outs=[rs.opt()])
            return rs

        def tanh_shard(rs, bias_sb):
            rs_sb = fpool.tile([B, J], F32, name="rs_sb", tag="rssb")
            nc.sync.dma_start(rs_sb, rs)
            pre = fpool.tile([B, J], F32, name="pre", tag="pre")
            nc.vector.tensor_tensor(pre, rs_sb, bias_sb, _ALU.add)
            h_f32 = hpool.tile([B, J], F32, name="h_f32", tag="hf32")
            nc.scalar.activation(h_f32, pre,
                                 mybir.ActivationFunctionType.Tanh,
                                 bias=0.0, scale=1.0 / SW)
            return h_f32

        def split_transpose(h_f32, st_next):
            # h_hi/h_lo are KS wide with zeroed tails so all 5 transposes are
            # uniform 128 cols and the stationary pad rows become zeros.
            h_hi = hpool.tile([B, KS], F16, name="h_hi", tag="hhi")
            nc.vector.tensor_copy(h_hi[:, 0:J], h_f32)
            nc.vector.tensor_scalar_mul(h_hi[:, J:KS], h_f32[:, 0:KS - J], 0.0)
            hsub = fpool.tile([B, J], F32, name="hsub", tag="hsub")
            nc.vector.tensor_tensor(hsub, h_f32, h_hi[:, 0:J], _ALU.subtract)
            h_lo = hpool.tile([B, KS], F16, name="h_lo", tag="hlo")
            nc.vector.tensor_scalar_mul(h_lo[:, 0:J], hsub, SLH)
            nc.vector.tensor_scalar_mul(h_lo[:, J:KS], h_f32[:, 0:KS - J], 0.0)
            for sel, src in ((0, h_lo), (1, h_hi)):
                for c in range(KT_S):
                    tp = tp_pool.tile([128, B], F16, name="tp", tag="tp")
                    nc.tensor.matmul(tp, src[:, 128 * c:128 * (c + 1)],
                                     ident[0:B, 0:B], is_transpose=True,
                                     skip_group_check=True)
                    nc.vector.tensor_copy(
                        st_next[:, c * B2 + sel * B:c * B2 + (sel + 1) * B],
                        tp)

        # ---- time loop ----
        for t in range(T):
            # layer 0
            rs0 = apply_rs(
                0, lambda pcs, half: mm_half(pcs, "w0", h0st, half,
                                             True, True))
            h0_f32 = tanh_shard(rs0, xproj_sb)
            h0st_n = new_stationary("h0")
            split_transpose(h0_f32, h0st_n)

            # layer 1: W_hh1 (old h1) first in each half to cover RS0 latency
            def l1_half(pcs, half):
                mm_half(pcs, "w1h", h1st, half, True, False)
                mm_half(pcs, "w1i", h0st_n, half, False, True)

            rs1 = apply_rs(1, l1_half)
            h1_f32 = tanh_shard(rs1, b1_sb)
            if t >= T - T_out:
                nc.sync.dma_start(ys[0:B, t - (T - T_out), 0:J], h1_f32)
            h1st_n = new_stationary("h1")
            split_transpose(h1_f32, h1st_n)

            h0st, h1st = h0st_n, h1st_n


# ------------------------------------------------------------------
# host side
# ------------------------------------------------------------------

def _pad_to(x, n, axis):
    w = [(0, 0)] * x.ndim
    w[axis] = (0, n - x.shape[axis])
    return np.pad(x, w)


def prep_inputs(hidden, W_ih0, W_hh0, b_ih0, b_hh0, W_ih1, W_hh1, b_ih1, b_hh1):
    f32 = np.float32
    hidden = np.asarray(hidden, f32)
    xproj_full = _pad_to(np.asarray(b_ih0, f32) + np.asarray(b_hh0, f32), K, 0) * f32(SW)
    b1_full = _pad_to(np.asarray(b_ih1, f32) + np.asarray(b_hh1, f32), K, 0) * f32(SW)

    def wsplit(W):
        WT = np.asarray(W, f32).T.copy()          # [k, j]
        WT = _pad_to(_pad_to(WT, K, 0), K, 1) * f32(SW)
        hi = WT.astype(np.float16)
        lo = (WT - hi.astype(f32)).astype(np.float16)
        return hi, lo

    w0_hi, w0_lo = wsplit(W_hh0)
    w1i_hi, w1i_lo = wsplit(W_ih1)
    w1h_hi, w1h_lo = wsplit(W_hh1)

    def hsplit(h):
        """-> [K, 2B]: per k row [lo(64) | hi(64)]."""
        hT = _pad_to(np.asarray(h, f32), K, 1).T.copy()
        hi = hT.astype(np.float16)
        lo = ((hT - hi.astype(f32)) * f32(SLH)).astype(np.float16)
        return np.concatenate([lo[:, None, :], hi[:, None, :]],
                              axis=1).reshape(K, B2)

    h0t = hsplit(hidden[0])
    h1t = hsplit(hidden[1])
    ident = np.eye(128, dtype=np.float16)

    def kslice(a, m):
        return np.ascontiguousarray(_pad_to(a[J * m:J * (m + 1)], KS, 0))

    in_maps = []
    for m in range(NCORES):
        js = slice(J * m, J * (m + 1))
        in_maps.append({
            "w0_hi": kslice(w0_hi, m), "w0_lo": kslice(w0_lo, m),
            "w1i_hi": kslice(w1i_hi, m), "w1i_lo": kslice(w1i_lo, m),
            "w1h_hi": kslice(w1h_hi, m), "w1h_lo": kslice(w1h_lo, m),
            "h0t": kslice(h0t, m), "h1t": kslice(h1t, m),
            "xproj": np.ascontiguousarray(
                np.broadcast_to(xproj_full[js], (B, J))),
            "b1": np.ascontiguousarray(np.broadcast_to(b1_full[js], (B, J))),
            "ident": ident,
        })
    return in_maps


_IN_SPECS = [
    ("w0_hi", [KS, K], np.float16), ("w0_lo", [KS, K], np.float16),
    ("w1i_hi", [KS, K], np.float16), ("w1i_lo", [KS, K], np.float16),
    ("w1h_hi", [KS, K], np.float16), ("w1h_lo", [KS, K], np.float16),
    ("h0t", [KS, B2], np.float16), ("h1t", [KS, B2], np.float16),
    ("xproj", [B, J], np.float32), ("b1", [B, J], np.float32),
    ("ident", [128, 128], np.float16),
]

_BUILD_CACHE = {}


def build_nc(T, T_out=None):
    key = (T, T_out)
    if key in _BUILD_CACHE:
        return _BUILD_CACHE[key]
    T_out = T if T_out is None else T_out
    nc = bacc.Bacc("TRN2", target_bir_lowering=False, debug=False,
                   num_devices=NCORES)
    ins = {name: nc.dram_tensor(name, shape, mybir.dt.from_np(np.dtype(dt)),
                                kind="ExternalInput").ap()
           for name, shape, dt in _IN_SPECS}
    outs = {"ys": nc.dram_tensor("ys", [B, T_out, J], mybir.dt.float32,
                                 kind="ExternalOutput").ap()}
    with tile.TileContext(nc) as tc:
        build(tc, outs, ins, T, T_out)
    nc.compile()
    _BUILD_CACHE[key] = nc
    return nc


def kernel(**inputs):
    inputs = {k: np.asarray(v) for k, v in inputs.items()}
    in_maps = prep_inputs(**inputs)
    nc = build_nc(T_FULL)
    trace = bool(int(os.environ.get("BASS_PROFILE", "0")))
    res = run_bass_kernel_spmd(nc, in_maps, core_ids=list(range(NCORES)),
                               trace=trace)
    kernel._last = res
    ys = np.concatenate([res.results[m]["ys"] for m in range(NCORES)], axis=2)
    return np.ascontiguousarray(ys[:, :, :H_REAL]).astype(np.float32)
